# revision 13
# baseline (speedup 1.0000x reference)
"""GAT layer (nn_GATlayer) Trainium2 Bass kernel, 8-core row-parallel.

All-fp32 device pipeline:
  phase 1 (attention, rows sharded): proj replicated from host-transposed
  nodesT; s_src/s_tgt via folded vectors va=W@a_src, vt=W@a_tgt;
  scores = lrelu(s_src+s_tgt) - wd*dist + wb*bond + mask(degree) using the
  min(D*1e9, D) trick; e=exp(scores) with ACT row-sum accum; out_pre =
  (e @ proj)/Z via PE transposes; ELU -> out_elu shard.
  AllGather shards -> E4; AllToAll 32-col slices -> my o^T columns.
  phase 2 (upd): sim = o@o.T from strided-copy o^T layouts; u2' =
  sigmoid(sim)*where(D>0,X,0); row layernorm (eps shift dropped);
  v = -(u2'-mean)*rstd; per-strip AllToAll of v; upd = v + recv^T via PE
  transposes + fused add; host only concatenates and view-permutes.
"""
import numpy as np

N = 4096
F = 256
NCORES = 8
ROWS = N // NCORES          # 512 rows per core
NSTRIP = ROWS // 128        # 4 strips of 128 rows
CW = 512                    # chunk width
NCH = N // CW               # 8 chunks per strip
BIG = 1.0e9
LRELU = 0.2
LN_EPS = 1e-05

_CACHE = {}


def _build():
    import concourse.bass as bass
    from concourse import bacc
    import concourse.tile as tile
    import concourse.mybir as mybir
    from contextlib import ExitStack

    f32 = mybir.dt.float32
    Alu = mybir.AluOpType
    Act = mybir.ActivationFunctionType

    nc = bacc.Bacc("TRN2", target_bir_lowering=False, debug=False,
                   num_devices=NCORES)

    d_rows = nc.dram_tensor("d_rows", [ROWS, N], f32, kind="ExternalInput")
    x_rows = nc.dram_tensor("x_rows", [ROWS, N], f32, kind="ExternalInput")
    b_rows = nc.dram_tensor("b_rows", [ROWS, N], f32, kind="ExternalInput")
    nodes_t = nc.dram_tensor("nodes_t", [F, N], f32, kind="ExternalInput")
    nodes_my_t = nc.dram_tensor("nodes_my_t", [F, ROWS], f32, kind="ExternalInput")
    nodes_my = nc.dram_tensor("nodes_my", [ROWS, F], f32, kind="ExternalInput")
    w_mat = nc.dram_tensor("w_mat", [F, F], f32, kind="ExternalInput")
    wt_mat = nc.dram_tensor("wt_mat", [F, F], f32, kind="ExternalInput")
    a_vec = nc.dram_tensor("a_vec", [F, 2], f32, kind="ExternalInput")
    wvec = nc.dram_tensor("wvec", [1, 2], f32, kind="ExternalInput")
    ident_in = nc.dram_tensor("ident_in", [128, 128], f32, kind="ExternalInput")

    out_elu = nc.dram_tensor("out_elu", [ROWS, F], f32, kind="ExternalOutput")
    dbg_tb = nc.dram_tensor("dbg_tb", [128, 512], f32, kind="ExternalOutput")
    dbg_sck = nc.dram_tensor("dbg_sck", [128, 512], f32, kind="ExternalOutput")
    dbg_e = nc.dram_tensor("dbg_e", [128, 512], f32, kind="ExternalOutput")
    dbg_z = nc.dram_tensor("dbg_z", [128, 4], f32, kind="ExternalOutput")
    dbg_o1 = nc.dram_tensor("dbg_o1", [128, 512], f32, kind="ExternalOutput")
    dbg_lck = nc.dram_tensor("dbg_lck", [128, 512], f32, kind="ExternalOutput")
    dbg_t1 = nc.dram_tensor("dbg_t1", [128, 512], f32, kind="ExternalOutput")
    dbg_t2 = nc.dram_tensor("dbg_t2", [128, 512], f32, kind="ExternalOutput")
    dbg_ss = nc.dram_tensor("dbg_ss", [128, 4], f32, kind="ExternalOutput")
    dbg_pj = nc.dram_tensor("dbg_pj", [128, 256], f32, kind="ExternalOutput")
    upd_rows = nc.dram_tensor("upd_rows", [ROWS, N], f32, kind="ExternalOutput")

    rg = [list(range(NCORES))]
    CPC = F // NCORES  # 32 columns per core in the column exchange

    with tile.TileContext(nc) as tc, ExitStack() as top:
        persist = top.enter_context(tc.tile_pool(name="persist", bufs=1))
        dram = top.enter_context(tc.tile_pool(name="dram", bufs=1, space="DRAM"))

        m1v = persist.tile([128, NSTRIP, N], f32)    # m1; later holds v
        ident = persist.tile([128, 128], f32)
        wv = persist.tile([128, 2], f32)
        s_src_cols = persist.tile([128, NSTRIP], f32)
        zrec = persist.tile([128, NSTRIP], f32)
        stats = persist.tile([128, NSTRIP, 4], f32)
        eps_t = persist.tile([128, 1], f32)
        nc.vector.memset(eps_t, LN_EPS)

        nc.sync.dma_start(out=ident, in_=ident_in[:, :])
        wv_src = wvec[:, :]
        nc.sync.dma_start(out=wv, in_=bass.AP(
            tensor=wv_src.tensor, offset=wv_src.offset,
            ap=[[0, 128]] + wv_src.ap[1:]))

        agb_in = dram.tile([ROWS, F], f32)
        agb_out = dram.tile([N, F], f32)
        colb_in = dram.tile([N, CPC], f32)
        colb_out = dram.tile([N, CPC], f32)
        a2a_in = [dram.tile([NCORES * 128, ROWS], f32, tag=f"a2ain{s}",
                            name=f"a2ain{s}")
                  for s in range(NSTRIP)]
        a2a_out = [dram.tile([NCORES * 128, ROWS], f32, tag=f"a2aout{s}",
                             name=f"a2aout{s}")
                   for s in range(NSTRIP)]
        srow_d = dram.tile([1, N], f32)
        va_d = dram.tile([1, F], f32)

        # ================= setup + phase 1 =================
        with ExitStack() as ph1:
            span1 = ph1.enter_context(tc.tile_pool(name="span1", bufs=1))
            proj = span1.tile([128, N // 128, F], f32)
            tb_bc = span1.tile([128, N], f32)

            with tc.tile_pool(name="setup", bufs=1) as setup, \
                 tc.tile_pool(name="setup_ps", bufs=2, space="PSUM") as setup_ps:
                ndt = setup.tile([128, 2, N], f32)
                nmt = setup.tile([128, 2, ROWS], f32)
                wm = setup.tile([128, 2, F], f32)
                wtm = setup.tile([128, 2, F], f32)
                av = setup.tile([128, 2, 2], f32)

                nc.sync.dma_start(out=ndt, in_=nodes_t[:, :].rearrange(
                    "(a p) n -> p a n", p=128))
                nc.sync.dma_start(out=nmt, in_=nodes_my_t[:, :].rearrange(
                    "(a p) n -> p a n", p=128))
                nc.sync.dma_start(out=wm, in_=w_mat[:, :].rearrange(
                    "(a p) f -> p a f", p=128))
                nc.sync.dma_start(out=wtm, in_=wt_mat[:, :].rearrange(
                    "(a p) f -> p a f", p=128))
                nc.sync.dma_start(out=av, in_=a_vec[:, :].rearrange(
                    "(a p) k -> p a k", p=128))

                # va|vt = W @ [a_src|a_tgt] -> vsb[:, fi_chunk, k]
                vsb = setup.tile([128, 2, 2], f32)
                for a in range(2):
                    pv = setup_ps.tile([128, 2], f32, tag="pv")
                    for b in range(2):
                        for k in range(2):
                            nc.tensor.matmul(
                                pv[:, k:k + 1],
                                wtm[:, b, a * 128:(a + 1) * 128],
                                av[:, b, k:k + 1],
                                start=(b == 0), stop=(b == 1))
                    nc.scalar.copy(vsb[:, a, :], pv)

                for j in range(N // 128):
                    pp = setup_ps.tile([128, F], f32, tag="pp")
                    for a in range(2):
                        nc.tensor.matmul(pp, ndt[:, a, j * 128:(j + 1) * 128],
                                         wm[:, a, :], start=(a == 0), stop=(a == 1))
                    nc.scalar.copy(proj[:, j, :], pp)

                nc.sync.dma_start(out=dbg_pj[:, :], in_=proj[:, 0, :])
                srow = setup.tile([1, N], f32)
                for q in range(NCH):
                    pr = setup_ps.tile([1, CW], f32, tag="pr")
                    for a in range(2):
                        nc.tensor.matmul(pr, vsb[:, a, 1:2],
                                         ndt[:, a, q * CW:(q + 1) * CW],
                                         start=(a == 0), stop=(a == 1))
                    nc.scalar.copy(srow[:, q * CW:(q + 1) * CW], pr)
                nc.sync.dma_start(out=srow_d, in_=srow)

                # va as a [1, F] row, then broadcast and DVE-accumulate
                var_row = setup.tile([1, F], f32)
                pvr = setup_ps.tile([1, F], f32, tag="pvr")
                for b in range(2):
                    nc.tensor.matmul(pvr, av[:, b, 0:1], wtm[:, b, :],
                                     start=(b == 0), stop=(b == 1))
                nc.scalar.copy(var_row, pvr)
                nc.sync.dma_start(out=va_d, in_=var_row)
                va_b = setup.tile([128, F], f32)
                vd_ap = va_d.opt()
                nc.sync.dma_start(out=va_b, in_=bass.AP(
                    tensor=vd_ap.tensor, offset=vd_ap.offset,
                    ap=[[0, 128]] + vd_ap.ap[1:]))
                nmn = setup.tile([128, NSTRIP, F], f32)
                nc.sync.dma_start(out=nmn, in_=nodes_my[:, :].rearrange(
                    "(s p) f -> p s f", p=128))
                for s in range(NSTRIP):
                    dump_s = setup.tile([128, F], f32, tag="dump_s")
                    nc.vector.scalar_tensor_tensor(
                        out=dump_s, in0=nmn[:, s, :], scalar=1.0, in1=va_b,
                        op0=Alu.mult, op1=Alu.mult,
                        accum_out=s_src_cols[:, s:s + 1])

                sd_ap = srow_d.opt()
                nc.sync.dma_start(out=tb_bc, in_=bass.AP(
                    tensor=sd_ap.tensor, offset=sd_ap.offset,
                    ap=[[0, 128]] + sd_ap.ap[1:]))

            with tc.tile_pool(name="p1c", bufs=2) as p1c, \
                 tc.tile_pool(name="p1s", bufs=2) as p1s, \
                 tc.tile_pool(name="p1_pt", bufs=2, space="PSUM") as p1_pt, \
                 tc.tile_pool(name="p1_av", bufs=2, space="PSUM") as p1_av:
                for s in range(NSTRIP):
                    rs = slice(s * 128, (s + 1) * 128)
                    avp = p1_av.tile([128, F], f32, tag="avp")
                    zaccs = p1s.tile([128, NCH], f32, tag="zaccs")
                    for q in range(NCH):
                        cs = slice(q * CW, (q + 1) * CW)
                        dck = p1c.tile([128, CW], f32, tag="dck")
                        xck = p1c.tile([128, CW], f32, tag="xck")
                        bck = p1c.tile([128, CW], f32, tag="bck")
                        nc.sync.dma_start(out=dck, in_=d_rows[rs, cs])
                        nc.sync.dma_start(out=xck, in_=x_rows[rs, cs])
                        nc.sync.dma_start(out=bck, in_=b_rows[rs, cs])
                        o1 = p1c.tile([128, CW], f32, tag="o1")
                        nc.scalar.activation(o1, tb_bc[:, cs], Act.Identity,
                                             bias=s_src_cols[:, s:s + 1], scale=1.0)
                        lck = p1c.tile([128, CW], f32, tag="lck")
                        nc.vector.scalar_tensor_tensor(
                            out=lck, in0=o1, scalar=LRELU, in1=o1,
                            op0=Alu.mult, op1=Alu.max)
                        t1 = p1c.tile([128, CW], f32, tag="t1")
                        nc.vector.scalar_tensor_tensor(
                            out=t1, in0=xck, scalar=wv[:, 0:1], in1=lck,
                            op0=Alu.mult, op1=Alu.add)
                        gck = p1c.tile([128, CW], f32, tag="gck")
                        nc.vector.scalar_tensor_tensor(
                            out=gck, in0=dck, scalar=BIG, in1=dck,
                            op0=Alu.mult, op1=Alu.min)
                        t2 = p1c.tile([128, CW], f32, tag="t2")
                        nc.vector.scalar_tensor_tensor(
                            out=t2, in0=bck, scalar=wv[:, 1:2], in1=gck,
                            op0=Alu.mult, op1=Alu.add)
                        sck = p1c.tile([128, CW], f32, tag="sck")
                        nc.vector.tensor_add(sck, t1, t2)
                        nc.vector.scalar_tensor_tensor(
                            out=m1v[:, s, cs], in0=dck, scalar=0.0, in1=xck,
                            op0=Alu.is_gt, op1=Alu.mult)
                        eck = p1c.tile([128, CW], f32, tag="eck")
                        nc.scalar.activation(eck, sck, Act.Exp, bias=0.0,
                                             scale=1.0, accum_out=zaccs[:, q:q + 1])
                        if s == 0 and q == 0:
                            nc.sync.dma_start(out=dbg_tb[:, :], in_=tb_bc[:, 0:512])
                            nc.sync.dma_start(out=dbg_sck[:, :], in_=sck)
                            nc.sync.dma_start(out=dbg_e[:, :], in_=eck)
                            nc.sync.dma_start(out=dbg_o1[:, :], in_=o1)
                            nc.sync.dma_start(out=dbg_lck[:, :], in_=lck)
                            nc.sync.dma_start(out=dbg_t1[:, :], in_=t1)
                            nc.sync.dma_start(out=dbg_t2[:, :], in_=t2)
                            nc.sync.dma_start(out=dbg_ss[:, :], in_=s_src_cols)
                        pt = p1_pt.tile([128, CW], f32, tag="pt")
                        for k in range(CW // 128):
                            nc.tensor.transpose(
                                pt[:, k * 128:(k + 1) * 128],
                                eck[:, k * 128:(k + 1) * 128], ident)
                        etc = p1c.tile([128, CW // 128, 128], f32, tag="etc")
                        nc.scalar.copy(etc, pt.rearrange("p (k c) -> p k c",
                                                         k=CW // 128))
                        for k in range(CW // 128):
                            cc = q * (CW // 128) + k
                            nc.tensor.matmul(avp, etc[:, k, :], proj[:, cc, :],
                                             start=(cc == 0),
                                             stop=(cc == N // 128 - 1))
                    zsum = p1s.tile([128, 1], f32, tag="zsum")
                    nc.vector.tensor_reduce(out=zsum, in_=zaccs,
                                            axis=mybir.AxisListType.X, op=Alu.add)
                    nc.vector.reciprocal(out=zrec[:, s:s + 1], in_=zsum)
                    if s == NSTRIP - 1:
                        nc.sync.dma_start(out=dbg_z[:, :], in_=zrec)
                    orow = p1s.tile([128, F], f32, tag="orow")
                    nc.vector.tensor_scalar_mul(orow, avp, zrec[:, s:s + 1])
                    mn = p1s.tile([128, F], f32, tag="mn")
                    nc.vector.tensor_scalar_min(mn, orow, 0.0)
                    expm = p1s.tile([128, F], f32, tag="expm")
                    nc.scalar.activation(expm, mn, Act.Exp, bias=0.0, scale=1.0)
                    rl = p1s.tile([128, F], f32, tag="rl")
                    nc.vector.tensor_scalar_max(rl, orow, 0.0)
                    oel = p1s.tile([128, F], f32, tag="oel")
                    nc.vector.scalar_tensor_tensor(
                        out=oel, in0=expm, scalar=-1.0, in1=rl,
                        op0=Alu.add, op1=Alu.add)
                    nc.sync.dma_start(out=out_elu[rs, :], in_=oel)
                    nc.sync.dma_start(out=agb_in[rs, :], in_=oel)

        # ================= exchange + phase 2 =================
        nc.gpsimd.collective_compute(
            "AllGather", mybir.AluOpType.bypass, replica_groups=rg,
            ins=[agb_in.opt()], outs=[agb_out.opt()])
        for j in range(NCORES):
            nc.sync.dma_start(
                out=colb_in[j * ROWS:(j + 1) * ROWS, :],
                in_=agb_in.opt()[:, j * CPC:(j + 1) * CPC])
        nc.gpsimd.collective_compute(
            "AllToAll", mybir.AluOpType.bypass, replica_groups=rg,
            ins=[colb_in.opt()], outs=[colb_out.opt()])

        with ExitStack() as ph2:
            span2 = ph2.enter_context(tc.tile_pool(name="span2", bufs=1))
            oT = span2.tile([128, 2, N], f32)
            oTmy = span2.tile([128, 2, ROWS], f32)
            u2p = span2.tile([128, NSTRIP, N], f32)

            with tc.tile_pool(name="otb", bufs=3) as otb:
                for b in range(16):
                    for r in range(2):
                        e4t = otb.tile([128, F], f32, tag="e4t")
                        nc.sync.dma_start(
                            out=e4t,
                            in_=agb_out.opt()[256 * b + 128 * r:
                                              256 * b + 128 * (r + 1), :])
                        ov = oT[:, r, :].rearrange("p (a b2) -> p a b2", b2=16)
                        nc.scalar.copy(ov[:, :, b], e4t)
                        cbt = otb.tile([128, CPC], f32, tag="cbt")
                        nc.sync.dma_start(
                            out=cbt,
                            in_=colb_out.opt()[256 * b + 128 * r:
                                               256 * b + 128 * (r + 1), :])
                        omv = oTmy[:, r, :].rearrange("p (a b2) -> p a b2", b2=16)
                        nc.vector.tensor_copy(omv[:, :, b], cbt)

            with tc.tile_pool(name="p2", bufs=2) as p2, \
                 tc.tile_pool(name="p2_ps", bufs=3, space="PSUM") as p2_ps:
                for s in range(NSTRIP):
                    saccs = p2.tile([128, NCH], f32, tag="saccs")
                    qaccs = p2.tile([128, NCH], f32, tag="qaccs")
                    for q in range(NCH):
                        cs = slice(q * CW, (q + 1) * CW)
                        sp = p2_ps.tile([128, CW], f32, tag="sp")
                        for u in range(2):
                            nc.tensor.matmul(
                                sp, oTmy[:, u, s * 128:(s + 1) * 128],
                                oT[:, u, cs], start=(u == 0), stop=(u == 1))
                        uck = p2.tile([128, CW], f32, tag="uck")
                        nc.scalar.activation(uck, sp, Act.Sigmoid,
                                             bias=0.0, scale=1.0)
                        nc.vector.scalar_tensor_tensor(
                            out=u2p[:, s, cs], in0=uck, scalar=1.0,
                            in1=m1v[:, s, cs], op0=Alu.mult, op1=Alu.mult,
                            accum_out=saccs[:, q:q + 1])
                        dump = p2.tile([128, CW], f32, tag="dump")
                        nc.scalar.activation(dump, u2p[:, s, cs], Act.Square,
                                             bias=0.0, scale=1.0,
                                             accum_out=qaccs[:, q:q + 1])
                    ssum = p2.tile([128, 1], f32, tag="ssum")
                    nc.vector.tensor_reduce(out=ssum, in_=saccs,
                                            axis=mybir.AxisListType.X, op=Alu.add)
                    qsum = p2.tile([128, 1], f32, tag="qsum")
                    nc.vector.tensor_reduce(out=qsum, in_=qaccs,
                                            axis=mybir.AxisListType.X, op=Alu.add)
                    nc.vector.tensor_copy(stats[:, s, 0:1], ssum)
                    nc.vector.tensor_copy(stats[:, s, 1:2], qsum)

                for s in range(NSTRIP):
                    mean = p2.tile([128, 1], f32, tag="mean")
                    nc.vector.tensor_scalar_mul(mean, stats[:, s, 0:1], 1.0 / N)
                    msq = p2.tile([128, 1], f32, tag="msq")
                    nc.vector.tensor_mul(msq, mean, mean)
                    var = p2.tile([128, 1], f32, tag="var")
                    nc.vector.scalar_tensor_tensor(
                        out=var, in0=stats[:, s, 1:2], scalar=1.0 / N, in1=msq,
                        op0=Alu.mult, op1=Alu.subtract)
                    lnv = p2.tile([128, 1], f32, tag="lnv")
                    nc.scalar.activation(lnv, var, Act.Ln, bias=eps_t, scale=1.0)
                    rstd = p2.tile([128, 1], f32, tag="rstd")
                    nc.scalar.activation(rstd, lnv, Act.Exp, bias=0.0, scale=-0.5)
                    nc.vector.tensor_scalar_mul(stats[:, s, 2:3], rstd, -1.0)
                    nc.vector.tensor_mul(stats[:, s, 3:4], mean, rstd)

                for s in range(NSTRIP):
                    # v = -(u2'-mean)*rstd = u2'*(-rstd) + mean*rstd
                    nc.vector.tensor_scalar(
                        out=m1v[:, s, :], in0=u2p[:, s, :],
                        scalar1=stats[:, s, 2:3], scalar2=stats[:, s, 3:4],
                        op0=Alu.mult, op1=Alu.add)
                    for j in range(NCORES):
                        nc.sync.dma_start(
                            out=a2a_in[s][j * 128:(j + 1) * 128, :],
                            in_=m1v[:, s, j * ROWS:(j + 1) * ROWS])

        for s in range(NSTRIP):
            nc.gpsimd.collective_compute(
                "AllToAll", mybir.AluOpType.bypass, replica_groups=rg,
                ins=[a2a_in[s].opt()], outs=[a2a_out[s].opt()])

        with tc.tile_pool(name="p3", bufs=2) as p3, \
             tc.tile_pool(name="p3_ps", bufs=3, space="PSUM") as p3_ps:
            for t in range(NSTRIP):
                rvs = []
                for s in range(NSTRIP):
                    rv = p3.tile([128, NCORES, 128], f32, tag=f"rv{s}")
                    nc.sync.dma_start(
                        out=rv,
                        in_=a2a_out[s].opt()[:, t * 128:(t + 1) * 128].rearrange(
                            "(j p) c -> p j c", p=128))
                    rvs.append(rv)
                for j in range(NCORES):
                    fp = p3_ps.tile([128, CW], f32, tag="fp")
                    for s in range(NSTRIP):
                        nc.tensor.transpose(
                            fp[:, s * 128:(s + 1) * 128], rvs[s][:, j, :], ident)
                    fo = p3.tile([128, CW], f32, tag="fo")
                    nc.vector.scalar_tensor_tensor(
                        out=fo, in0=m1v[:, t, j * CW:(j + 1) * CW],
                        scalar=1.0, in1=fp, op0=Alu.mult, op1=Alu.add)
                    nc.sync.dma_start(
                        out=upd_rows[t * 128:(t + 1) * 128, j * CW:(j + 1) * CW],
                        in_=fo)

    nc.compile()
    return nc


def _get_nc():
    if "nc" not in _CACHE:
        _CACHE["nc"] = _build()
    return _CACHE["nc"]


def kernel(nodes_features, degree_matrix, edges_features_distance,
           edges_features_bond, proj_param, a_src, a_tgt, w_dist, w_bond,
           cutoff=0):
    from concourse.bass_utils import run_bass_kernel_spmd

    nc = _get_nc()

    nodes = np.ascontiguousarray(np.asarray(nodes_features, dtype=np.float32))
    D = np.ascontiguousarray(np.asarray(degree_matrix, dtype=np.float32))
    X = np.ascontiguousarray(np.asarray(edges_features_distance, dtype=np.float32))
    B = np.ascontiguousarray(np.asarray(edges_features_bond, dtype=np.float32))
    W = np.ascontiguousarray(np.asarray(proj_param, dtype=np.float32)[0])
    asrc = np.asarray(a_src, dtype=np.float32)[0, :, 0]
    atgt = np.asarray(a_tgt, dtype=np.float32)[0, :, 0]
    wd = float(np.asarray(w_dist, dtype=np.float32).reshape(-1)[0])
    wb = float(np.asarray(w_bond, dtype=np.float32).reshape(-1)[0])

    nodes_t = np.ascontiguousarray(nodes.T)
    wt = np.ascontiguousarray(W.T)
    avec = np.ascontiguousarray(np.stack([asrc, atgt], axis=1))
    wvec_np = np.array([[-wd, wb]], dtype=np.float32)
    ident = np.eye(128, dtype=np.float32)

    in_maps = []
    for i in range(NCORES):
        rs = slice(i * ROWS, (i + 1) * ROWS)
        in_maps.append({
            "d_rows": np.ascontiguousarray(D[rs]),
            "x_rows": np.ascontiguousarray(X[rs]),
            "b_rows": np.ascontiguousarray(B[rs]),
            "nodes_t": nodes_t,
            "nodes_my_t": np.ascontiguousarray(nodes_t[:, rs]),
            "nodes_my": np.ascontiguousarray(nodes[rs]),
            "w_mat": W,
            "wt_mat": wt,
            "a_vec": avec,
            "wvec": wvec_np,
            "ident_in": ident,
        })

    res = run_bass_kernel_spmd(nc, in_maps, core_ids=list(range(NCORES)))

    e4 = np.concatenate([res.results[i]["out_elu"] for i in range(NCORES)], axis=0)
    upd = np.concatenate([res.results[i]["upd_rows"] for i in range(NCORES)], axis=0)
    # reference view-permute: out[16a+b, j] = e4[256b+j, a]
    out = np.ascontiguousarray(
        e4.reshape(16, 256, 256).transpose(2, 0, 1).reshape(N, F))
    return out, upd


# revision 15
# speedup vs baseline: 1.2420x; 1.2420x over previous
"""GAT layer (nn_GATlayer) Trainium2 Bass kernel, 8-core row-parallel.

All-fp32 device pipeline:
  phase 1 (attention, rows sharded): proj replicated from host-transposed
  nodesT; s_src/s_tgt via folded vectors va=W@a_src, vt=W@a_tgt;
  scores = lrelu(s_src+s_tgt) - wd*dist + wb*bond + mask(degree) using the
  min(D*1e9, D) trick; e=exp(scores) with ACT row-sum accum; out_pre =
  (e @ proj)/Z via PE transposes; ELU -> out_elu shard.
  AllGather shards -> E4; AllToAll 32-col slices -> my o^T columns.
  phase 2 (upd): sim = o@o.T from strided-copy o^T layouts; u2' =
  sigmoid(sim)*where(D>0,X,0); row layernorm (eps shift dropped);
  v = -(u2'-mean)*rstd; per-strip AllToAll of v; upd = v + recv^T via PE
  transposes + fused add; host only concatenates and view-permutes.
"""
import numpy as np

N = 4096
F = 256
NCORES = 8
ROWS = N // NCORES          # 512 rows per core
NSTRIP = ROWS // 128        # 4 strips of 128 rows
CW = 512                    # chunk width
NCH = N // CW               # 8 chunks per strip
BIG = 1.0e9
LRELU = 0.2
LN_EPS = 1e-05

_CACHE = {}


def _build():
    import concourse.bass as bass
    from concourse import bacc
    import concourse.tile as tile
    import concourse.mybir as mybir
    from contextlib import ExitStack

    f32 = mybir.dt.float32
    Alu = mybir.AluOpType
    Act = mybir.ActivationFunctionType

    nc = bacc.Bacc("TRN2", target_bir_lowering=False, debug=False,
                   num_devices=NCORES)

    d_rows = nc.dram_tensor("d_rows", [ROWS, N], f32, kind="ExternalInput")
    x_rows = nc.dram_tensor("x_rows", [ROWS, N], f32, kind="ExternalInput")
    b_rows = nc.dram_tensor("b_rows", [ROWS, N], f32, kind="ExternalInput")
    nodes_t = nc.dram_tensor("nodes_t", [F, N], f32, kind="ExternalInput")
    nodes_my_t = nc.dram_tensor("nodes_my_t", [F, ROWS], f32, kind="ExternalInput")
    nodes_my = nc.dram_tensor("nodes_my", [ROWS, F], f32, kind="ExternalInput")
    w_mat = nc.dram_tensor("w_mat", [F, F], f32, kind="ExternalInput")
    wt_mat = nc.dram_tensor("wt_mat", [F, F], f32, kind="ExternalInput")
    a_vec = nc.dram_tensor("a_vec", [F, 2], f32, kind="ExternalInput")
    wvec = nc.dram_tensor("wvec", [1, 2], f32, kind="ExternalInput")
    ident_in = nc.dram_tensor("ident_in", [128, 128], f32, kind="ExternalInput")

    out_elu = nc.dram_tensor("out_elu", [ROWS, F], f32, kind="ExternalOutput")
    upd_rows = nc.dram_tensor("upd_rows", [ROWS, N], f32, kind="ExternalOutput")

    rg = [list(range(NCORES))]
    CPC = F // NCORES  # 32 columns per core in the column exchange

    with tile.TileContext(nc) as tc, ExitStack() as top:
        persist = top.enter_context(tc.tile_pool(name="persist", bufs=1))
        dram = top.enter_context(tc.tile_pool(name="dram", bufs=1, space="DRAM"))

        m1v = persist.tile([128, NSTRIP, N], f32)    # m1; later holds v
        ident = persist.tile([128, 128], f32)
        wv = persist.tile([128, 2], f32)
        s_src_cols = persist.tile([128, NSTRIP], f32)
        zrec = persist.tile([128, NSTRIP], f32)
        stats = persist.tile([128, NSTRIP, 4], f32)
        eps_t = persist.tile([128, 1], f32)
        nc.vector.memset(eps_t, LN_EPS)

        nc.sync.dma_start(out=ident, in_=ident_in[:, :])
        wv_src = wvec[:, :]
        nc.sync.dma_start(out=wv, in_=bass.AP(
            tensor=wv_src.tensor, offset=wv_src.offset,
            ap=[[0, 128]] + wv_src.ap[1:]))

        agb_in = dram.tile([ROWS, F], f32)
        agb_out = dram.tile([N, F], f32)
        colb_in = dram.tile([N, CPC], f32)
        colb_out = dram.tile([N, CPC], f32)
        a2a_in = [dram.tile([NCORES * 128, ROWS], f32, tag=f"a2ain{s}",
                            name=f"a2ain{s}")
                  for s in range(NSTRIP)]
        a2a_out = [dram.tile([NCORES * 128, ROWS], f32, tag=f"a2aout{s}",
                             name=f"a2aout{s}")
                   for s in range(NSTRIP)]
        srow_d = dram.tile([1, N], f32)
        va_d = dram.tile([1, F], f32)

        # ================= setup + phase 1 =================
        with ExitStack() as ph1:
            span1 = ph1.enter_context(tc.tile_pool(name="span1", bufs=1))
            proj = span1.tile([128, N // 128, F], f32)
            tb_bc = span1.tile([128, N], f32)

            with tc.tile_pool(name="setup", bufs=1) as setup, \
                 tc.tile_pool(name="setup_ps", bufs=2, space="PSUM") as setup_ps:
                ndt = setup.tile([128, 2, N], f32)
                nmt = setup.tile([128, 2, ROWS], f32)
                wm = setup.tile([128, 2, F], f32)
                wtm = setup.tile([128, 2, F], f32)
                av = setup.tile([128, 2, 2], f32)

                nc.sync.dma_start(out=ndt, in_=nodes_t[:, :].rearrange(
                    "(a p) n -> p a n", p=128))
                nc.sync.dma_start(out=nmt, in_=nodes_my_t[:, :].rearrange(
                    "(a p) n -> p a n", p=128))
                nc.sync.dma_start(out=wm, in_=w_mat[:, :].rearrange(
                    "(a p) f -> p a f", p=128))
                nc.sync.dma_start(out=wtm, in_=wt_mat[:, :].rearrange(
                    "(a p) f -> p a f", p=128))
                nc.sync.dma_start(out=av, in_=a_vec[:, :].rearrange(
                    "(a p) k -> p a k", p=128))

                # va|vt = W @ [a_src|a_tgt] -> vsb[:, fi_chunk, k]
                vsb = setup.tile([128, 2, 2], f32)
                for a in range(2):
                    pv = setup_ps.tile([128, 2], f32, tag="pv")
                    for b in range(2):
                        for k in range(2):
                            nc.tensor.matmul(
                                pv[:, k:k + 1],
                                wtm[:, b, a * 128:(a + 1) * 128],
                                av[:, b, k:k + 1],
                                start=(b == 0), stop=(b == 1))
                    nc.scalar.copy(vsb[:, a, :], pv)

                for j in range(N // 128):
                    pp = setup_ps.tile([128, F], f32, tag="pp")
                    for a in range(2):
                        nc.tensor.matmul(pp, ndt[:, a, j * 128:(j + 1) * 128],
                                         wm[:, a, :], start=(a == 0), stop=(a == 1))
                    nc.scalar.copy(proj[:, j, :], pp)

                srow = setup.tile([1, N], f32)
                for q in range(NCH):
                    pr = setup_ps.tile([1, CW], f32, tag="pr")
                    for a in range(2):
                        nc.tensor.matmul(pr, vsb[:, a, 1:2],
                                         ndt[:, a, q * CW:(q + 1) * CW],
                                         start=(a == 0), stop=(a == 1))
                    nc.scalar.copy(srow[:, q * CW:(q + 1) * CW], pr)
                nc.sync.dma_start(out=srow_d, in_=srow)

                # va as a [1, F] row, then broadcast and DVE-accumulate
                var_row = setup.tile([1, F], f32)
                pvr = setup_ps.tile([1, F], f32, tag="pvr")
                for b in range(2):
                    nc.tensor.matmul(pvr, av[:, b, 0:1], wtm[:, b, :],
                                     start=(b == 0), stop=(b == 1))
                nc.scalar.copy(var_row, pvr)
                nc.sync.dma_start(out=va_d, in_=var_row)
                va_b = setup.tile([128, F], f32)
                vd_ap = va_d.opt()
                nc.sync.dma_start(out=va_b, in_=bass.AP(
                    tensor=vd_ap.tensor, offset=vd_ap.offset,
                    ap=[[0, 128]] + vd_ap.ap[1:]))
                nmn = setup.tile([128, NSTRIP, F], f32)
                nc.sync.dma_start(out=nmn, in_=nodes_my[:, :].rearrange(
                    "(s p) f -> p s f", p=128))
                for s in range(NSTRIP):
                    dump_s = setup.tile([128, F], f32, tag="dump_s")
                    nc.vector.scalar_tensor_tensor(
                        out=dump_s, in0=nmn[:, s, :], scalar=1.0, in1=va_b,
                        op0=Alu.mult, op1=Alu.mult,
                        accum_out=s_src_cols[:, s:s + 1])

                sd_ap = srow_d.opt()
                nc.sync.dma_start(out=tb_bc, in_=bass.AP(
                    tensor=sd_ap.tensor, offset=sd_ap.offset,
                    ap=[[0, 128]] + sd_ap.ap[1:]))

            with tc.tile_pool(name="p1c", bufs=2) as p1c, \
                 tc.tile_pool(name="p1s", bufs=2) as p1s, \
                 tc.tile_pool(name="p1_pt", bufs=2, space="PSUM") as p1_pt, \
                 tc.tile_pool(name="p1_av", bufs=2, space="PSUM") as p1_av:
                for s in range(NSTRIP):
                    rs = slice(s * 128, (s + 1) * 128)
                    avp = p1_av.tile([128, F], f32, tag="avp")
                    zaccs = p1s.tile([128, NCH], f32, tag="zaccs")
                    for q in range(NCH):
                        cs = slice(q * CW, (q + 1) * CW)
                        dck = p1c.tile([128, CW], f32, tag="dck")
                        xck = p1c.tile([128, CW], f32, tag="xck")
                        bck = p1c.tile([128, CW], f32, tag="bck")
                        nc.sync.dma_start(out=dck, in_=d_rows[rs, cs])
                        nc.sync.dma_start(out=xck, in_=x_rows[rs, cs])
                        nc.sync.dma_start(out=bck, in_=b_rows[rs, cs])
                        o1 = p1c.tile([128, CW], f32, tag="o1")
                        nc.scalar.activation(o1, tb_bc[:, cs], Act.Identity,
                                             bias=s_src_cols[:, s:s + 1], scale=1.0)
                        lck = p1c.tile([128, CW], f32, tag="lck")
                        nc.vector.scalar_tensor_tensor(
                            out=lck, in0=o1, scalar=LRELU, in1=o1,
                            op0=Alu.mult, op1=Alu.max)
                        t1 = p1c.tile([128, CW], f32, tag="t1")
                        nc.vector.scalar_tensor_tensor(
                            out=t1, in0=xck, scalar=wv[:, 0:1], in1=lck,
                            op0=Alu.mult, op1=Alu.add)
                        gck = p1c.tile([128, CW], f32, tag="gck")
                        nc.vector.scalar_tensor_tensor(
                            out=gck, in0=dck, scalar=BIG, in1=dck,
                            op0=Alu.mult, op1=Alu.min)
                        t2 = p1c.tile([128, CW], f32, tag="t2")
                        nc.vector.scalar_tensor_tensor(
                            out=t2, in0=bck, scalar=wv[:, 1:2], in1=gck,
                            op0=Alu.mult, op1=Alu.add)
                        sck = p1c.tile([128, CW], f32, tag="sck")
                        nc.vector.tensor_add(sck, t1, t2)
                        nc.vector.scalar_tensor_tensor(
                            out=m1v[:, s, cs], in0=dck, scalar=0.0, in1=xck,
                            op0=Alu.is_gt, op1=Alu.mult)
                        eck = p1c.tile([128, CW], f32, tag="eck")
                        nc.scalar.activation(eck, sck, Act.Exp, bias=0.0,
                                             scale=1.0, accum_out=zaccs[:, q:q + 1])
                        pt = p1_pt.tile([128, CW], f32, tag="pt")
                        for k in range(CW // 128):
                            nc.tensor.transpose(
                                pt[:, k * 128:(k + 1) * 128],
                                eck[:, k * 128:(k + 1) * 128], ident)
                        etc = p1c.tile([128, CW // 128, 128], f32, tag="etc")
                        nc.scalar.copy(etc, pt.rearrange("p (k c) -> p k c",
                                                         k=CW // 128))
                        for k in range(CW // 128):
                            cc = q * (CW // 128) + k
                            nc.tensor.matmul(avp, etc[:, k, :], proj[:, cc, :],
                                             start=(cc == 0),
                                             stop=(cc == N // 128 - 1))
                    zsum = p1s.tile([128, 1], f32, tag="zsum")
                    nc.vector.tensor_reduce(out=zsum, in_=zaccs,
                                            axis=mybir.AxisListType.X, op=Alu.add)
                    nc.vector.reciprocal(out=zrec[:, s:s + 1], in_=zsum)
                    orow = p1s.tile([128, F], f32, tag="orow")
                    nc.vector.tensor_scalar_mul(orow, avp, zrec[:, s:s + 1])
                    mn = p1s.tile([128, F], f32, tag="mn")
                    nc.vector.tensor_scalar_min(mn, orow, 0.0)
                    expm = p1s.tile([128, F], f32, tag="expm")
                    nc.scalar.activation(expm, mn, Act.Exp, bias=0.0, scale=1.0)
                    rl = p1s.tile([128, F], f32, tag="rl")
                    nc.vector.tensor_scalar_max(rl, orow, 0.0)
                    oel = p1s.tile([128, F], f32, tag="oel")
                    nc.vector.scalar_tensor_tensor(
                        out=oel, in0=expm, scalar=-1.0, in1=rl,
                        op0=Alu.add, op1=Alu.add)
                    nc.sync.dma_start(out=out_elu[rs, :], in_=oel)
                    nc.sync.dma_start(out=agb_in[rs, :], in_=oel)

        # ================= exchange + phase 2 =================
        nc.gpsimd.collective_compute(
            "AllGather", mybir.AluOpType.bypass, replica_groups=rg,
            ins=[agb_in.opt()], outs=[agb_out.opt()])
        for j in range(NCORES):
            nc.sync.dma_start(
                out=colb_in[j * ROWS:(j + 1) * ROWS, :],
                in_=agb_in.opt()[:, j * CPC:(j + 1) * CPC])
        nc.gpsimd.collective_compute(
            "AllToAll", mybir.AluOpType.bypass, replica_groups=rg,
            ins=[colb_in.opt()], outs=[colb_out.opt()])

        with ExitStack() as ph2:
            span2 = ph2.enter_context(tc.tile_pool(name="span2", bufs=1))
            oT = span2.tile([128, 2, N], f32)
            oTmy = span2.tile([128, 2, ROWS], f32)
            u2p = span2.tile([128, NSTRIP, N], f32)

            with tc.tile_pool(name="otb", bufs=3) as otb:
                for b in range(16):
                    for r in range(2):
                        e4t = otb.tile([128, F], f32, tag="e4t")
                        nc.sync.dma_start(
                            out=e4t,
                            in_=agb_out.opt()[256 * b + 128 * r:
                                              256 * b + 128 * (r + 1), :])
                        ov = oT[:, r, :].rearrange("p (a b2) -> p a b2", b2=16)
                        nc.scalar.copy(ov[:, :, b], e4t)
                        cbt = otb.tile([128, CPC], f32, tag="cbt")
                        nc.sync.dma_start(
                            out=cbt,
                            in_=colb_out.opt()[256 * b + 128 * r:
                                               256 * b + 128 * (r + 1), :])
                        omv = oTmy[:, r, :].rearrange("p (a b2) -> p a b2", b2=16)
                        nc.vector.tensor_copy(omv[:, :, b], cbt)

            with tc.tile_pool(name="p2", bufs=2) as p2, \
                 tc.tile_pool(name="p2_ps", bufs=3, space="PSUM") as p2_ps:
                for s in range(NSTRIP):
                    saccs = p2.tile([128, NCH], f32, tag="saccs")
                    qaccs = p2.tile([128, NCH], f32, tag="qaccs")
                    for q in range(NCH):
                        cs = slice(q * CW, (q + 1) * CW)
                        sp = p2_ps.tile([128, CW], f32, tag="sp")
                        for u in range(2):
                            nc.tensor.matmul(
                                sp, oTmy[:, u, s * 128:(s + 1) * 128],
                                oT[:, u, cs], start=(u == 0), stop=(u == 1))
                        uck = p2.tile([128, CW], f32, tag="uck")
                        nc.scalar.activation(uck, sp, Act.Sigmoid,
                                             bias=0.0, scale=1.0)
                        nc.vector.scalar_tensor_tensor(
                            out=u2p[:, s, cs], in0=uck, scalar=1.0,
                            in1=m1v[:, s, cs], op0=Alu.mult, op1=Alu.mult,
                            accum_out=saccs[:, q:q + 1])
                        dump = p2.tile([128, CW], f32, tag="dump")
                        nc.scalar.activation(dump, u2p[:, s, cs], Act.Square,
                                             bias=0.0, scale=1.0,
                                             accum_out=qaccs[:, q:q + 1])
                    ssum = p2.tile([128, 1], f32, tag="ssum")
                    nc.vector.tensor_reduce(out=ssum, in_=saccs,
                                            axis=mybir.AxisListType.X, op=Alu.add)
                    qsum = p2.tile([128, 1], f32, tag="qsum")
                    nc.vector.tensor_reduce(out=qsum, in_=qaccs,
                                            axis=mybir.AxisListType.X, op=Alu.add)
                    nc.vector.tensor_copy(stats[:, s, 0:1], ssum)
                    nc.vector.tensor_copy(stats[:, s, 1:2], qsum)

                for s in range(NSTRIP):
                    mean = p2.tile([128, 1], f32, tag="mean")
                    nc.vector.tensor_scalar_mul(mean, stats[:, s, 0:1], 1.0 / N)
                    msq = p2.tile([128, 1], f32, tag="msq")
                    nc.vector.tensor_mul(msq, mean, mean)
                    var = p2.tile([128, 1], f32, tag="var")
                    nc.vector.scalar_tensor_tensor(
                        out=var, in0=stats[:, s, 1:2], scalar=1.0 / N, in1=msq,
                        op0=Alu.mult, op1=Alu.subtract)
                    lnv = p2.tile([128, 1], f32, tag="lnv")
                    nc.scalar.activation(lnv, var, Act.Ln, bias=eps_t, scale=1.0)
                    rstd = p2.tile([128, 1], f32, tag="rstd")
                    nc.scalar.activation(rstd, lnv, Act.Exp, bias=0.0, scale=-0.5)
                    nc.vector.tensor_scalar_mul(stats[:, s, 2:3], rstd, -1.0)
                    nc.vector.tensor_mul(stats[:, s, 3:4], mean, rstd)

                for s in range(NSTRIP):
                    # v = -(u2'-mean)*rstd = u2'*(-rstd) + mean*rstd
                    nc.vector.tensor_scalar(
                        out=m1v[:, s, :], in0=u2p[:, s, :],
                        scalar1=stats[:, s, 2:3], scalar2=stats[:, s, 3:4],
                        op0=Alu.mult, op1=Alu.add)
                    for j in range(NCORES):
                        nc.sync.dma_start(
                            out=a2a_in[s][j * 128:(j + 1) * 128, :],
                            in_=m1v[:, s, j * ROWS:(j + 1) * ROWS])

        for s in range(NSTRIP):
            nc.gpsimd.collective_compute(
                "AllToAll", mybir.AluOpType.bypass, replica_groups=rg,
                ins=[a2a_in[s].opt()], outs=[a2a_out[s].opt()])

        with tc.tile_pool(name="p3", bufs=2) as p3, \
             tc.tile_pool(name="p3_ps", bufs=3, space="PSUM") as p3_ps:
            for t in range(NSTRIP):
                rvs = []
                for s in range(NSTRIP):
                    rv = p3.tile([128, NCORES, 128], f32, tag=f"rv{s}")
                    nc.sync.dma_start(
                        out=rv,
                        in_=a2a_out[s].opt()[:, t * 128:(t + 1) * 128].rearrange(
                            "(j p) c -> p j c", p=128))
                    rvs.append(rv)
                for j in range(NCORES):
                    fp = p3_ps.tile([128, CW], f32, tag="fp")
                    for s in range(NSTRIP):
                        nc.tensor.transpose(
                            fp[:, s * 128:(s + 1) * 128], rvs[s][:, j, :], ident)
                    fo = p3.tile([128, CW], f32, tag="fo")
                    nc.vector.scalar_tensor_tensor(
                        out=fo, in0=m1v[:, t, j * CW:(j + 1) * CW],
                        scalar=1.0, in1=fp, op0=Alu.mult, op1=Alu.add)
                    nc.sync.dma_start(
                        out=upd_rows[t * 128:(t + 1) * 128, j * CW:(j + 1) * CW],
                        in_=fo)

    nc.compile()
    return nc


def _get_nc():
    if "nc" not in _CACHE:
        _CACHE["nc"] = _build()
    return _CACHE["nc"]


def kernel(nodes_features, degree_matrix, edges_features_distance,
           edges_features_bond, proj_param, a_src, a_tgt, w_dist, w_bond,
           cutoff=0):
    from concourse.bass_utils import run_bass_kernel_spmd

    nc = _get_nc()

    nodes = np.ascontiguousarray(np.asarray(nodes_features, dtype=np.float32))
    D = np.ascontiguousarray(np.asarray(degree_matrix, dtype=np.float32))
    X = np.ascontiguousarray(np.asarray(edges_features_distance, dtype=np.float32))
    B = np.ascontiguousarray(np.asarray(edges_features_bond, dtype=np.float32))
    W = np.ascontiguousarray(np.asarray(proj_param, dtype=np.float32)[0])
    asrc = np.asarray(a_src, dtype=np.float32)[0, :, 0]
    atgt = np.asarray(a_tgt, dtype=np.float32)[0, :, 0]
    wd = float(np.asarray(w_dist, dtype=np.float32).reshape(-1)[0])
    wb = float(np.asarray(w_bond, dtype=np.float32).reshape(-1)[0])

    nodes_t = np.ascontiguousarray(nodes.T)
    wt = np.ascontiguousarray(W.T)
    avec = np.ascontiguousarray(np.stack([asrc, atgt], axis=1))
    wvec_np = np.array([[-wd, wb]], dtype=np.float32)
    ident = np.eye(128, dtype=np.float32)

    in_maps = []
    for i in range(NCORES):
        rs = slice(i * ROWS, (i + 1) * ROWS)
        in_maps.append({
            "d_rows": np.ascontiguousarray(D[rs]),
            "x_rows": np.ascontiguousarray(X[rs]),
            "b_rows": np.ascontiguousarray(B[rs]),
            "nodes_t": nodes_t,
            "nodes_my_t": np.ascontiguousarray(nodes_t[:, rs]),
            "nodes_my": np.ascontiguousarray(nodes[rs]),
            "w_mat": W,
            "wt_mat": wt,
            "a_vec": avec,
            "wvec": wvec_np,
            "ident_in": ident,
        })

    res = None
    last_err = None
    for _attempt in range(4):
        try:
            res = run_bass_kernel_spmd(nc, in_maps, core_ids=list(range(NCORES)))
            break
        except Exception as ex:  # transient device-unrecoverable states
            last_err = ex
            import time as _time
            _time.sleep(10)
    if res is None:
        raise last_err

    e4 = np.concatenate([res.results[i]["out_elu"] for i in range(NCORES)], axis=0)
    upd = np.concatenate([res.results[i]["upd_rows"] for i in range(NCORES)], axis=0)
    # reference view-permute: out[16a+b, j] = e4[256b+j, a]
    out = np.ascontiguousarray(
        e4.reshape(16, 256, 256).transpose(2, 0, 1).reshape(N, F))
    return out, upd
